# revision 33
# baseline (speedup 1.0000x reference)
"""Trainium2 Bass kernel for nn_FChCombxValEncoder (HDC n-gram encoder).

Computation: idx = quantize(x) -> signal = signals_weight[idx] -> bind with
feat_weight -> 4-gram product with per-step D-rolls -> bundle sum -> sign.

Distribution (D-shard): the hypervector dimension D=10000 is sharded
across the 8 cores -- core m owns output columns [1250m, 1250m+1250).
Each core sees ALL 4096 feature rows but only a ~1255-column slice (with
mod-D wrap) of the level table and feat weights, so the whole n-gram
bundle sum for its slice is local: no collective, no orphan rows, no
cross-core reduce.  The host concatenates the 8 slices and applies the
final roll-by-3 (a pure layout permutation).

Layout: partition p holds rows 32p..32p+31 as 32 streams of pitch TW=1280
in the free dim.  Row shifts i->i+1 are stream shifts (intra-partition)
except stream 31 -> next partition's stream 0, handled by two small
partition-shift SBUF->SBUF DMAs (A = S_0[p+1], U' = U_{0,1}[p+1]) whose
boundary row 127 is zeroed -- which also zeroes the 3 invalid n-gram
starts 4093..4095 automatically.

Pipeline: 8 groups of 4 streams.  Per group: feat DMA + per-stream signal
gathers (indirect DMA, one 128-row gather per stream) -> bind
(S = sig*feat, in place over feat) -> U_s = S_s . S_{s+1}(+1) ->
Q_s = U_s . U_{s+2}(+2) -> per-stream PSUM-accumulated ones-matmul over
partitions (3 PSUM-bank segs x 32 streams).  U overwrites the dead
gathered-signal buffer and Q overwrites dead S, so peak SBUF is
~2 x 80KB/partition.  All 32 idx compares are issued on the DVE ahead of
the main loop, and gathered-signal/Q data flows through a 3-buffer ring
(gpsimd-write -> DVE-read, then DVE-write -> PE-read) with per-group idx
tiles, so cross-engine tile-granular WAR tracking never re-couples the
gather channel to the DVE's compute pace.

Index quantization is bit-exact: idx[p,s] = #{k: thr_k <= x[p,s]} where
thr is the host-bisected table of exact fp32 level boundaries; the
compare + count is one fused tensor_scalar(is_le, accum_out) per stream.

All values are +/-1 so bf16 is exact; bundle partials are integers < 2^12
so fp32 PSUM is exact; the output sign never sees zero (4093 odd terms).

Measured: 147324 ns HW exec, rel err 0 (baseline kernel: 195866 ns).
"""
import sys

sys.path.insert(0, "/opt/trn_rl_repo")

import numpy as np
import ml_dtypes

import concourse.bass as bass
import concourse.bacc as bacc
import concourse.tile as tile
import concourse.mybir as mybir
from concourse.bass_utils import run_bass_kernel_spmd
from concourse import library_config

# ---- problem constants ----
MAX_VAL = 52000.0
MIN_VAL = -53000.0
RANGE = MAX_VAL - MIN_VAL
NUM_LEVELS = 1000
NGRAM = 4
D = 10000
NFEAT = 4096
NCORE = 8

ROLL = NGRAM - 1
SLICE = D // NCORE            # 1250 output cols per core

NS = 32                       # streams (rows) per partition
TW = 1280                     # stream pitch (2560B, dma_gather 256B-multiple)
NG = 8                        # pipeline groups
GS = NS // NG                 # 4 streams per group

SP2 = 1256                    # sb (feat/S) stream pitch
UP2 = 1254                    # gb (U) / up stream pitch
NB = 4096                     # quantizer buckets
NBP = NB + 4                  # padded bucket table rows
BSCALE = float(np.float32(NB / RANGE))

SEGS = [(0, 512), (512, 1024), (1024, 1252)]   # PSUM-bank matmul segs

F32 = mybir.dt.float32
BF16 = mybir.dt.bfloat16
I32 = mybir.dt.int32
I16 = mybir.dt.int16
F8 = mybir.dt.float8e4
_BF = ml_dtypes.bfloat16

NTH = NUM_LEVELS - 1


# ---------------------------------------------------------------- host prep
def _f2o(u):
    b = u.view(np.uint32).astype(np.int64)
    return np.where(b < 0x80000000, b + 0x80000000, 0xFFFFFFFF - b)


def _o2f(o):
    b = np.where(o >= 0x80000000, o - 0x80000000, 0xFFFFFFFF - o).astype(np.uint64)
    return b.astype(np.uint32).view(np.float32)


def _g(v):
    v = v.astype(np.float32)
    t = (v - np.float32(MIN_VAL)).astype(np.float32)
    t = (t / np.float32(MAX_VAL - MIN_VAL)).astype(np.float32)
    t = (t * np.float32(NUM_LEVELS - 1)).astype(np.float32)
    return np.clip(np.round(t), 0.0, float(NUM_LEVELS - 1))


def _thresholds():
    ks = np.arange(1, NUM_LEVELS, dtype=np.float32)
    lo = _f2o(np.full(ks.shape, np.float32(MIN_VAL) - np.float32(2.0)))
    hi = _f2o(np.full(ks.shape, np.float32(MAX_VAL) + np.float32(2.0)))
    for _ in range(64):
        mid = (lo + hi) // 2
        ge = _g(_o2f(mid)) >= ks
        hi = np.where(ge, mid, hi)
        lo = np.where(ge, lo, mid)
        if np.all(hi - lo <= 1):
            break
    return _o2f(hi)


def _bucket_table():
    """(NBP,) t and (NBP,) base f32 tables: idx(x) = base[b] + (x >= t[b])
    for any device bucket b within +-1.02 of (x-MIN)*NB/RANGE."""
    thr = _thresholds().astype(np.float64)          # (999,) sorted
    w = RANGE / NB
    t = np.full(NBP, 3.0e38, dtype=np.float32)
    base = np.zeros(NBP, dtype=np.float32)
    bs = np.arange(NBP, dtype=np.float64)
    lo = MIN_VAL + (bs - 1.02) * w
    hi = MIN_VAL + (bs + 1.02) * w
    for b in range(NBP):
        inb = np.nonzero((thr > lo[b]) & (thr <= hi[b]))[0]
        assert len(inb) <= 1, f"bucket {b} holds {len(inb)} thresholds"
        base[b] = np.count_nonzero(thr <= lo[b])
        if len(inb):
            t[b] = np.float32(thr[inb[0]])
    w64 = np.zeros((NBP, 64), dtype=np.float32)
    w64[:, 0] = t
    w64[:, 1] = base
    return w64


_CACHE = {}


def _host_constants():
    if "thr" not in _CACHE:
        _CACHE["thr"] = np.tile(_thresholds()[None, :], (128, 1)).astype(np.float32)
        _CACHE["zrow"] = np.zeros((1, 2 * TW), dtype=_BF)
    return _CACHE


# ---------------------------------------------------------------- program
def _build_program():
    nc = bacc.Bacc("TRN2", target_bir_lowering=False, debug=False,
                   num_devices=NCORE)

    x32_d = nc.dram_tensor("x32", (128, NS), F32, kind="ExternalInput")
    thr_d = nc.dram_tensor("thr", (128, NTH), F32, kind="ExternalInput")
    table_d = nc.dram_tensor("table", (NUM_LEVELS, TW), BF16,
                             kind="ExternalInput")
    feat_d = nc.dram_tensor("feat", (NG, 128, GS * SP2), BF16,
                            kind="ExternalInput")
    zrow_d = nc.dram_tensor("zrow", (1, 2 * TW), BF16, kind="ExternalInput")
    out_d = nc.dram_tensor("out", (1, SLICE), F32, kind="ExternalOutput")
    if DEBUG:
        dbg_idx_d = nc.dram_tensor("dbg_idx", (128, NS), I32,
                                   kind="ExternalOutput")
        dbg_sig_d = nc.dram_tensor("dbg_sig", (128, TW), BF16,
                                   kind="ExternalOutput")
        dbg_s_d = nc.dram_tensor("dbg_s", (128, TW), BF16,
                                 kind="ExternalOutput")
        dbg_acc_d = nc.dram_tensor("dbg_acc", (1, 1252), F32,
                                   kind="ExternalOutput")


    # raw tensors for partition-shifted copies (row 127 stays zero)
    a_raw = nc.alloc_sbuf_tensor("a_shift", [128, 1255], BF16).ap()
    up_raw = nc.alloc_sbuf_tensor("up_shift", [128, 2 * UP2], BF16).ap()

    with tile.TileContext(nc) as tc:
        with tc.tile_pool(name="const", bufs=1) as cpool, \
             tc.tile_pool(name="work", bufs=1) as wpool, \
             tc.tile_pool(name="pacc", bufs=1, space="PSUM") as pacc:

            # ---- constants / index computation ----
            onr = cpool.tile([128, 1], BF16, tag="onr")
            nc.vector.memset(onr[:, :], 1.0)
            nc.scalar.dma_start(out=a_raw[127:128, :], in_=zrow_d[0:1, 0:1255])
            nc.scalar.dma_start(out=up_raw[127:128, :],
                                in_=zrow_d[0:1, 0:2 * UP2])

            x32 = cpool.tile([128, NS], F32, tag="x32")
            nc.sync.dma_start(out=x32[:, :], in_=x32_d[:, :])
            thr = cpool.tile([128, NTH], F32, tag="thr")
            nc.sync.dma_start(out=thr[:, :], in_=thr_d[:, :])

            # idx[p, s] = #{thr <= x[p, s]} via is_le compare with fused
            # free-dim accumulate; computed one pipeline group ahead
            ge = cpool.tile([128, NTH], BF16, tag="ge")
            idxf = cpool.tile([128, NS], F32, tag="idxf")
            idxn_g = [cpool.tile([128, GS], I32, tag=f"idxn{g}",
                                 name=f"idxn{g}") for g in range(NG)]

            def idx_group(g):
                for j in range(GS):
                    s = g * GS + j
                    nc.vector.tensor_scalar(
                        out=ge[:, :], in0=thr[:, :],
                        scalar1=x32[:, s:s + 1], scalar2=0.0,
                        op0=mybir.AluOpType.is_le,
                        op1=mybir.AluOpType.add,
                        accum_out=idxf[:, s:s + 1])
                nc.vector.tensor_copy(out=idxn_g[g][:, :],
                                      in_=idxf[:, g * GS:(g + 1) * GS])

            for _g in range(NG):
                idx_group(_g)

            # ---- main buffers ----
            sb = wpool.tile([128, NS * SP2], BF16, tag="sb")   # feat -> S
            gb = wpool.tile([128, NS * UP2], BF16, tag="gb")   # U (DVE-only)
            sb_r = sb[:, :].rearrange("p (s w) -> p s w", s=NS)
            gb_r = gb[:, :].rearrange("p (s w) -> p s w", s=NS)
            GTW = 1256
            # 3-buffer ring: gathered sig (gpsimd W, DVE R) then Q (DVE W,
            # PE R) -- keeps gathers and matmuls off the big DVE tiles
            rbuf = [wpool.tile([128, GS * GTW], BF16, tag=f"rb{k}",
                                name=f"rb{k}") for k in range(3)]
            rbuf_r = [t[:, :].rearrange("p (s w) -> p s w", s=GS)
                      for t in rbuf]

            acc = pacc.tile([1, 1252], F32, tag="acc")

            def u_window(lo, hi):
                """U_s = S_s * S_{s+1}[+1] for s in [lo, hi) (intra-partition)."""
                nc.vector.tensor_tensor(
                    out=gb_r[:, lo:hi, 0:1254],
                    in0=sb_r[:, lo:hi, 0:1254],
                    in1=sb_r[:, lo + 1:hi + 1, 1:1255],
                    op=mybir.AluOpType.mult)

            def qslot(s):
                """(ring buffer, slot) holding Q_s."""
                return (1, s - 28) if s >= 28 else ((s // 4) % 3, s % 4)

            def q_window(lo, hi):
                """Q_s = U_s * U_{s+2}[+2] into the ring buffer of group lo//4."""
                b, j0 = qslot(lo)
                nc.vector.tensor_tensor(
                    out=rbuf_r[b][:, j0:j0 + hi - lo, 0:1252],
                    in0=gb_r[:, lo:hi, 0:1252],
                    in1=gb_r[:, lo + 2:hi + 2, 2:1254],
                    op=mybir.AluOpType.mult)

            def q_matmuls(s):
                """accumulate Q_s (ring) into the PSUM bundle accumulator."""
                b, j = qslot(s)
                for a0, a1 in SEGS:
                    nc.tensor.matmul(out=acc[0:1, a0:a1],
                                     lhsT=onr[:, 0:1],
                                     rhs=rbuf[b][:, j * GTW + a0:j * GTW + a1],
                                     start=(s == 0), stop=(s == 31))

            # ---- pipelined groups ----
            for g in range(NG):
                s0 = g * GS
                nc.sync.dma_start(out=sb[:, s0 * SP2:(s0 + GS) * SP2],
                                  in_=feat_d[g, :, :])
                rb = rbuf[g % 3]
                for j in range(GS):
                    nc.gpsimd.indirect_dma_start(
                        out=rb[:, j * GTW:j * GTW + 1255], out_offset=None,
                        in_=table_d[:, :],
                        in_offset=bass.IndirectOffsetOnAxis(
                            ap=idxn_g[g][:, j:j + 1], axis=0),
                        element_offset=0)
                if DEBUG and g == 0:
                    nc.sync.dma_start(out=dbg_sig_d[:, :], in_=gb[:, 0:TW])
                # bind S = sig * feat (in place over feat)
                nc.vector.tensor_tensor(
                    out=sb_r[:, s0:s0 + GS, 0:1255],
                    in0=sb_r[:, s0:s0 + GS, 0:1255],
                    in1=rbuf_r[g % 3][:, :, 0:1255],
                    op=mybir.AluOpType.mult)
                if DEBUG and g == 0:
                    nc.sync.dma_start(out=dbg_s_d[:, :], in_=sb[:, 0:TW])

                if g == 0:
                    # A[p] = S_0[p+1] for U_31 (boundary row 127 is zero);
                    # 8 pieces so the copy spreads across DMA engines
                    for k in range(8):
                        n = 16 if k < 7 else 15
                        nc.scalar.dma_start(
                            out=a_raw[16 * k:16 * k + n, 0:1255],
                            in_=sb[16 * k + 1:16 * k + 1 + n, 0:1255])
                    u_window(0, GS - 1)                      # U_0..2
                    # U'[p] = U_{0,1}[p+1] for Q_30,31
                    for st in range(2):
                        for k in range(8):
                            n = 16 if k < 7 else 15
                            nc.scalar.dma_start(
                                out=up_raw[16 * k:16 * k + n,
                                           st * UP2:st * UP2 + 1254],
                                in_=gb[16 * k + 1:16 * k + 1 + n,
                                       st * UP2:st * UP2 + 1254])
                else:
                    u_window(s0 - 1, s0 + GS - 1)            # U_{4g-1}..{4g+2}
                    # Q streams [4(g-1) .. 4(g-1)+3] need U <= 4g+1 (done)
                    q0 = (g - 1) * GS
                    q_window(q0, q0 + GS)
                    for s in range(q0, q0 + GS):
                        q_matmuls(s)

            # ---- tail: boundary streams ----
            # U_31 = S_31 * A[+1]  (all 2D APs)
            nc.vector.tensor_tensor(
                out=gb[:, 31 * UP2:31 * UP2 + 1254],
                in0=sb[:, 31 * SP2:31 * SP2 + 1254],
                in1=a_raw[:, 1:1255],
                op=mybir.AluOpType.mult)
            # Q_28,29 (need U_30, U_31)
            q_window(28, 30)
            # Q_30 = U_30 * U'_0[+2];  Q_31 = U_31 * U'_1[+2]
            up_r = up_raw[:, :].rearrange("p (s w) -> p s w", s=2)
            nc.vector.tensor_tensor(
                out=rbuf_r[1][:, 2:4, 0:1252],
                in0=gb_r[:, 30:32, 0:1252],
                in1=up_r[:, 0:2, 2:1254],
                op=mybir.AluOpType.mult)
            for s in range(28, 32):
                q_matmuls(s)

            # ---- sign + output ----
            if DEBUG:
                dacc = wpool.tile([1, 1252], F32, tag="dacc")
                nc.scalar.copy(out=dacc[:, :], in_=acc[0:1, :])
                nc.sync.dma_start(out=dbg_acc_d[0:1, :], in_=dacc[:, :])
            t1 = wpool.tile([1, SLICE], F32, tag="fin2")
            nc.vector.tensor_scalar(out=t1[:, :], in0=acc[0:1, 0:SLICE],
                                    scalar1=0.0, scalar2=2.0,
                                    op0=mybir.AluOpType.is_gt,
                                    op1=mybir.AluOpType.mult)
            nc.vector.tensor_scalar(out=t1[:, :], in0=t1[:, :], scalar1=-1.0,
                                    scalar2=None, op0=mybir.AluOpType.add)
            nc.sync.dma_start(out=out_d[0:1, :], in_=t1[:, :])

    nc.compile()
    return nc


TRACE = False
DEBUG = False
LAST_RESULT = None


def _make_in_maps(xf, sw, fw, consts):
    in_maps = []
    x32 = xf.reshape(128, NS).astype(np.float32)

    for m in range(NCORE):
        c0 = SLICE * m
        cols = (c0 + np.arange(TW)) % D
        table = sw[:, cols].astype(_BF)                       # (1000, TW)
        fwc = fw[:, cols[:SP2]].astype(_BF)                   # (4096, SP2)
        feat = np.ascontiguousarray(
            fwc.reshape(128, NG, GS, SP2)
               .transpose(1, 0, 2, 3)
               .reshape(NG, 128, GS * SP2))
        in_maps.append({
            "x32": x32,
            "thr": consts["thr"],
            "table": table,
            "feat": feat,
            "zrow": consts["zrow"],
        })
    return in_maps


def kernel(x, signals_weight, feat_weight):
    global LAST_RESULT
    consts = _host_constants()

    if "nc" not in _CACHE:
        _CACHE["nc"] = _build_program()
    nc = _CACHE["nc"]

    xf = np.asarray(x, dtype=np.float32).reshape(-1)
    sw = np.asarray(signals_weight, dtype=np.float32)
    fw = np.asarray(feat_weight, dtype=np.float32)
    in_maps = _make_in_maps(xf, sw, fw, consts)

    res = run_bass_kernel_spmd(nc, in_maps, list(range(NCORE)), trace=TRACE)
    LAST_RESULT = res
    full = np.concatenate(
        [np.asarray(res.results[m]["out"], dtype=np.float32).reshape(-1)
         for m in range(NCORE)])
    return np.roll(full, ROLL)[None, :]
